# revision 41
# baseline (speedup 1.0000x reference)
"""Trainium2 Bass kernel for nn_AdaptiveExpertSystem (MoE, E=8, top-2).

Expert-parallel + pairwise tensor-parallel design. The host computes the
(cheap) router on CPU and uses it as the sharding function. Experts are
paired heavy+light (LPT), one pair per pair of cores: both cores of a
pair receive ALL tokens routed to either expert (pre-normalized by the
LN the router already computed, pre-transposed to feature-major), but
each core holds only HALF of the pair's w1/w2 along the intermediate
dimension I. Each core computes partial MLP outputs for all the pair's
tokens at the PE bf16 matmul roofline; the host sums the two halves and
scatter-adds with the top-2 combine weights (the unshard step).

This halves the load-imbalance penalty: per-core columns are
(n_a + n_b) ~ 2 * mean instead of max_e n_e.

ln_g/ln_b are folded into w1/b1 on the host so all experts share the
plain LN. All input DMAs ride one queue (gpsimd) in exact consumption
order — with 8 cores pulling 17MB of weights each, HBM is the binding
constraint at the head, so global ordering by need-time beats queue
parallelism.
"""
import numpy as np
import ml_dtypes

import concourse.bass as bass
import concourse.tile as tile
from concourse import bacc, mybir
from concourse.bass_utils import run_bass_kernel_spmd

N_CORES = 8
B, L, D, I, E = 2, 2048, 1024, 4096, 8
NTOK = B * L
KD = D // 128       # 8  d-tiles (contraction of mm1)
NI = I // 128       # 32 i-tiles total; each core runs NIH = 16
NIH = NI // 2
ND = D // 128       # 8  output d-tiles
LN_EPS = 1e-5

F32 = mybir.dt.float32
BF16 = mybir.dt.bfloat16
BF = ml_dtypes.bfloat16

_CACHE = {}


def _chunks_one(n, first):
    # chunk widths for one expert's n columns; every chunk in [232, 512]
    # keeps LDWEIGHTS hidden and fits a PSUM bank
    ws = []
    rest = n
    if first and n > 488:
        ws.append(256)
        rest = n - 256
    nch = max(1, (rest + 447) // 448)
    base = rest // nch
    extra = rest - base * nch
    for j in range(nch):
        ws.append(base + (1 if j < extra else 0))
    return ws


def _chunk_table(n1, n2):
    """[(expert_slot, col_offset, width), ...] covering [0,n1)+[n1,n1+n2)."""
    out = []
    o = 0
    for sl, n in ((0, n1), (1, n2)):
        for w in _chunks_one(n, first=(sl == 0)):
            out.append((sl, o, w))
            o += w
    return out


def build_nc(n1, n2):
    cap = n1 + n2
    chunks = _chunk_table(n1, n2)

    nc = bacc.Bacc(None, num_devices=N_CORES)
    xt_ps = [nc.declare_dram_parameter(f"xt{j}", [128, KD, w], BF16,
                                       isOutput=False)
             for j, (_, _, w) in enumerate(chunks)]
    # per expert-slot halves: w1 [slot][128, NIH, KD, 128],
    # w2 [slot][128, ND, NIH, 128]
    w1_p = nc.declare_dram_parameter("w1", [2, NIH, 128, KD, 128], BF16,
                                     isOutput=False)
    w2_p = nc.declare_dram_parameter("w2", [2, ND, 128, NIH, 128], BF16,
                                     isOutput=False)
    b1_p = nc.declare_dram_parameter("b1", [128, 2, NIH], F32, isOutput=False)
    out_p = nc.declare_dram_parameter("out", [ND, 128, cap], BF16,
                                      isOutput=True)

    AF = mybir.ActivationFunctionType

    from contextlib import ExitStack
    with tile.TileContext(nc) as tc, ExitStack() as ctx:
        ep = ctx.enter_context
        xntp = ep(tc.tile_pool(name="xnt", bufs=1))
        w1pool = ep(tc.tile_pool(name="w1p", bufs=1))
        w2pool = ep(tc.tile_pool(name="w2p", bufs=1))
        b1pool = ep(tc.tile_pool(name="b1p", bufs=1))
        h1pool = ep(tc.tile_pool(name="h1p", bufs=1))
        h2pool = ep(tc.tile_pool(name="h2p", bufs=2))
        ps1 = ep(tc.tile_pool(name="ps1", bufs=4, space="PSUM"))
        ps2 = ep(tc.tile_pool(name="ps2", bufs=4, space="PSUM"))

        xnT = xntp.tile([128, KD, cap], BF16)
        b1sb = b1pool.tile([128, 2, NIH], F32)
        nc.scalar.dma_start(out=b1sb, in_=b1_p[:])
        w1sb = w1pool.tile([128, 2, NIH, KD, 128], BF16)
        w2sb = w2pool.tile([128, 2, ND, NIH, 128], BF16)

        # consumption-ordered single-queue input DMAs: chunk0 tokens
        # (k-split) -> w1[slot0] -> chunk1 tokens -> w2[slot0] -> rest of
        # slot0 tokens -> w1[slot1] -> slot1 tokens -> w2[slot1]
        slot0_rest = [j for j, (sl, _, _) in enumerate(chunks)
                      if j > 0 and sl == 0]
        slot1_js = [j for j, (sl, _, _) in enumerate(chunks) if sl == 1]

        def xt_dma(j):
            _, c0, w = chunks[j]
            nc.gpsimd.dma_start(out=xnT[:, :, c0:c0 + w], in_=xt_ps[j][:])

        _, c00, w00 = chunks[0]
        for kk in range(0, KD, 2):
            nc.gpsimd.dma_start(out=xnT[:, kk:kk + 2, c00:c00 + w00],
                                in_=xt_ps[0][:, kk:kk + 2, :])
        for i in range(NIH):
            nc.gpsimd.dma_start(out=w1sb[:, 0, i], in_=w1_p[0, i])
        if slot0_rest:
            xt_dma(slot0_rest[0])
        for d in range(ND):
            nc.gpsimd.dma_start(out=w2sb[:, 0, d], in_=w2_p[0, d])
        for j in slot0_rest[1:]:
            xt_dma(j)
        for i in range(NIH):
            nc.gpsimd.dma_start(out=w1sb[:, 1, i], in_=w1_p[1, i])
        for j in slot1_js:
            xt_dma(j)
        for d in range(ND):
            nc.gpsimd.dma_start(out=w2sb[:, 1, d], in_=w2_p[1, d])

        for ci, (sl, c0, w) in enumerate(chunks):
            # ---- mm1 + gelu -> h1 (this chunk, this expert slot) ----
            h1 = h1pool.tile([128, NIH, w], BF16, name=f"h1_{ci}", tag="h1",
                             bufs=1)
            for i in range(NIH):
                p1 = ps1.tile([128, w], F32, tag="p1", name=f"p1_{ci}_{i}")
                for k in range(KD):
                    nc.tensor.matmul(p1, lhsT=w1sb[:, sl, i, k],
                                     rhs=xnT[:, k, c0:c0 + w],
                                     start=(k == 0), stop=(k == KD - 1))
                nc.scalar.activation(out=h1[:, i], in_=p1, func=AF.Gelu,
                                     bias=b1sb[:, sl, i:i + 1], scale=1.0)
            # ---- mm2 -> partial h2 -> DMA out ----
            for d in range(ND):
                p2 = ps2.tile([128, w], F32, tag="p2", name=f"p2_{ci}_{d}")
                for i in range(NIH):
                    nc.tensor.matmul(p2, lhsT=w2sb[:, sl, d, i], rhs=h1[:, i],
                                     start=(i == 0), stop=(i == NIH - 1))
                h2 = h2pool.tile([128, w], BF16, tag="h2",
                                 name=f"h2_{ci}_{d}")
                nc.vector.tensor_copy(out=h2, in_=p2)
                nc.sync.dma_start(out=out_p[d][:, c0:c0 + w], in_=h2)

    nc.finalize()
    return nc


def _pack_w1h(w1e_half):
    # [d, i_half] -> [i_tile, p, k, m]; d = k*128 + p, i = it*128 + m
    t = w1e_half.reshape(KD, 128, NIH, 128)
    return np.ascontiguousarray(t.transpose(2, 1, 0, 3)).astype(BF)


def _pack_w2h(w2e_half):
    # [i_half, d] -> [d_tile, p, i_tile, m]; i = it*128 + p, d = dt*128 + m
    t = w2e_half.reshape(NIH, 128, ND, 128)
    return np.ascontiguousarray(t.transpose(2, 1, 0, 3)).astype(BF)


def kernel(**inputs) -> np.ndarray:
    x = np.asarray(inputs["hidden_states"], np.float32).reshape(NTOK, D)
    rn_g = np.asarray(inputs["rn_g"], np.float32)
    rn_b = np.asarray(inputs["rn_b"], np.float32)
    router_w = np.asarray(inputs["router_w"], np.float32)
    router_b = np.asarray(inputs["router_b"], np.float32)
    ln_g = np.asarray(inputs["ln_g"], np.float32)
    ln_b = np.asarray(inputs["ln_b"], np.float32)
    w1 = np.asarray(inputs["w1"], np.float32)
    b1 = np.asarray(inputs["b1"], np.float32)
    w2 = np.asarray(inputs["w2"], np.float32)
    b2 = np.asarray(inputs["b2"], np.float32)

    # ---- Router on host: this IS the sharding function ----
    m = x.mean(-1, keepdims=True)
    v = ((x - m) ** 2).mean(-1, keepdims=True)
    rstd = 1.0 / np.sqrt(v + LN_EPS)
    normed = (x - m) * rstd
    logits = (normed * rn_g + rn_b) @ router_w.T + router_b
    top2 = np.argsort(-logits, axis=-1, kind="stable")[:, :2]
    tv = np.take_along_axis(logits, top2, -1)
    tv = np.exp(tv - tv.max(-1, keepdims=True))
    tw = (tv / tv.sum(-1, keepdims=True)).astype(np.float32)

    idxs, wts = [], []
    for e in range(E):
        sel = (top2[:, 0] == e) | (top2[:, 1] == e)
        idx_e = np.nonzero(sel)[0]
        w_e = np.where(top2[idx_e, 0] == e, tw[idx_e, 0], tw[idx_e, 1])
        idxs.append(idx_e)
        wts.append(w_e.astype(np.float32))

    # LPT pairing: heaviest with lightest
    order = sorted(range(E), key=lambda e: -len(idxs[e]))
    pairs = [(order[i], order[E - 1 - i]) for i in range(E // 2)]
    # round counts to multiples of 8 for clean strides
    caps = {e: max(256, ((len(idxs[e]) + 7) // 8) * 8) for e in range(E)}
    n1g = max(caps[a] for a, b in pairs)
    n2g = max(caps[b] for a, b in pairs)

    normed_bf = normed.astype(BF)
    in_maps = []
    chunks = _chunk_table(n1g, n2g)
    for a, b_ in pairs:
        g = np.zeros((n1g + n2g, D), dtype=BF)
        g[:len(idxs[a])] = normed_bf[idxs[a]]
        g[n1g:n1g + len(idxs[b_])] = normed_bf[idxs[b_]]
        t = g.T.reshape(KD, 128, n1g + n2g).transpose(1, 0, 2)
        xts = {}
        for j, (sl, c0, w) in enumerate(chunks):
            xts[f"xt{j}"] = np.ascontiguousarray(t[:, :, c0:c0 + w])
        for half in range(2):
            ilo, ihi = half * (I // 2), (half + 1) * (I // 2)
            w1s = np.stack([_pack_w1h((ln_g[e][:, None] * w1[e])[:, ilo:ihi])
                            for e in (a, b_)])
            w2s = np.stack([_pack_w2h(w2[e][ilo:ihi, :]) for e in (a, b_)])
            b1s = np.stack(
                [(b1[e] + ln_b[e] @ w1[e])[ilo:ihi].reshape(NIH, 128).T
                 .astype(np.float32) for e in (a, b_)], axis=1)
            mp = {"w1": w1s, "w2": w2s,
                  "b1": np.ascontiguousarray(b1s)}
            mp.update(xts)
            in_maps.append(mp)

    key = (n1g, n2g)
    if key not in _CACHE:
        _CACHE[key] = build_nc(n1g, n2g)
    nc = _CACHE[key]
    res = run_bass_kernel_spmd(nc, in_maps, core_ids=list(range(N_CORES)))

    # ---- Unshard: sum I-halves, weighted scatter-add (top-2 combine) ----
    out = tw[:, 0:1] * b2[top2[:, 0]] + tw[:, 1:2] * b2[top2[:, 1]]
    cap = n1g + n2g
    for pi, (a, b_) in enumerate(pairs):
        h2a = np.asarray(res.results[2 * pi]["out"], dtype=np.float32)
        h2b = np.asarray(res.results[2 * pi + 1]["out"], dtype=np.float32)
        h2 = (h2a + h2b).reshape(D, cap).T      # [cap, D]
        out[idxs[a]] += wts[a][:, None] * h2[:len(idxs[a])]
        out[idxs[b_]] += wts[b_][:, None] * h2[n1g:n1g + len(idxs[b_])]
    return out.reshape(B, L, D).astype(np.float32)
